# revision 1
# baseline (speedup 1.0000x reference)
"""Trainium2 Bass kernel for nn_MinibatchDiscrimination1d.

  x [256,1024] f32, T [1024,64,32] f32
  M = (x @ T.reshape(1024, 2048)).reshape(256, 64, 32)
  l1[i,j,b] = sum_c |M[i,b,c] - M[j,b,c]|
  out = concat([x, sum_j exp(-l1) - 1], axis=1)   # [256, 1088]

Sharding: the B=64 dimension is split across 8 cores (8 b's per core).
Each core computes the full M slice for its 8 b's (tensor-parallel over
T's columns) and the exp-sum for all 256 rows on its b-slice; the x
columns are copied through the cores row-sharded.

Per-core layout: MT[g] = [128 partitions = (4 b x 32 c), 256 = rows] for
g in {0,1}. For each row i the abs-diff |MT - MT[:,i]| is needed summed
over c. Using |d| = 2*relu(d) - d, the sum becomes
  l1[i,j,b] = 2*sum_c relu(d) - colsum[b,j] + colsum[b,i]
so one DVE tensor_scalar (sub+max -> relu) per (i,g) feeds a PE matmul
with a ones-selector (value 2.0) that reduces c on the partition axis;
-colsum[b,j] is one extra matmul per 16-row block, and colsum[b,i] rides
the per-partition bias of the exp activation. Some (i,g) tiles compute
the same relu(d) on the ScalarE (Relu activation, negated bias column)
to balance DVE/ACT load; every slot uses the identical decomposition. The selector matmuls for a 16-row block pack
the PSUM tile as [128 = (16 i x 8 b), 256 = j] using four concurrent
32-column PE strips; one Exp activation with accum_out then yields
sum_j exp(-l1) for 128 (i,b) pairs at once.
"""

import os
import numpy as np
import ml_dtypes

N = 256
A_DIM = 1024
B = 64
C = 32
NCORES = 8
BPC = B // NCORES          # 8 b's per core
P = 128
NBLK = 16                  # 16 i-blocks of 16 rows
BLK = 16

ACT_SLOTS = int(os.environ.get("KERN_ACT_SLOTS", "6"))  # of 32 (i,g) slots per block on ACT
A_BUFS = int(os.environ.get("KERN_A_BUFS", "24"))
# benchmarking only: repeat phase 2 in a hardware loop to make its duration
# measurable above host dispatch noise (1 = plain kernel, used for grading)
REPEAT = int(os.environ.get("KERN_REPEAT", "1"))
GPS_SLOTS = int(os.environ.get("KERN_GPS_SLOTS", "0"))  # of 32, taken from DVE's share

_cache = {}


def _act_assign(s, t, g):
    """Which (s,t,g) slots of a block go to the ScalarE (Abs) instead of DVE."""
    idx = (s * 4 + t) * 2 + g   # 0..31
    return (idx * ACT_SLOTS) // 32 != ((idx + 1) * ACT_SLOTS) // 32


def _gps_assign(s, t, g):
    """Slots on GpSimd (relu path, same as DVE). Never overlaps _act_assign:
    counts from the other end of the index space."""
    if _act_assign(s, t, g):
        return False
    idx = 31 - ((s * 4 + t) * 2 + g)
    return (idx * GPS_SLOTS) // 32 != ((idx + 1) * GPS_SLOTS) // 32


def build():
    import concourse.bacc as bacc
    import concourse.tile as tile
    from concourse import mybir

    dt = mybir.dt
    A = mybir.AluOpType
    F = mybir.ActivationFunctionType

    nc = bacc.Bacc("TRN2", target_bir_lowering=False, debug=False)

    xT_d = nc.dram_tensor("xT", [A_DIM, N], dt.float32, kind="ExternalInput")
    t2g_d = nc.dram_tensor("t2g", [A_DIM, BPC * C], dt.float32, kind="ExternalInput")
    xrows_d = nc.dram_tensor("xrows", [N // NCORES, A_DIM], dt.float32, kind="ExternalInput")
    sel2_d = nc.dram_tensor("sel2", [P, 64], dt.bfloat16, kind="ExternalInput")
    selneg_d = nc.dram_tensor("selneg", [P, 16], dt.bfloat16, kind="ExternalInput")
    wpos8_d = nc.dram_tensor("wpos8", [BPC, P], dt.bfloat16, kind="ExternalInput")
    sel2w_d = nc.dram_tensor("sel2w", [P, P], dt.bfloat16, kind="ExternalInput")

    outb_d = nc.dram_tensor("out_b", [N, BPC], dt.float32, kind="ExternalOutput")
    outx_d = nc.dram_tensor("out_x", [N // NCORES, A_DIM], dt.float32, kind="ExternalOutput")

    with tile.TileContext(nc) as tc:
        with (
            tc.tile_pool(name="const", bufs=1) as const,
            tc.tile_pool(name="apool", bufs=A_BUFS) as apool,
            tc.tile_pool(name="epool", bufs=3) as epool,
            tc.tile_pool(name="ps_mt", bufs=2, space="PSUM") as ps_mt,
            tc.tile_pool(name="ps_l1", bufs=5, space="PSUM") as ps_l1,
            tc.tile_pool(name="ps_cs", bufs=1, space="PSUM") as ps_cs,
            tc.tile_pool(name="dram", bufs=1, space="DRAM") as dram,
        ):
            # ---- x row-slice passthrough (independent of everything) ----
            xr = const.tile([N // NCORES, A_DIM], dt.float32)
            nc.sync.dma_start(out=xr, in_=xrows_d.ap())
            nc.sync.dma_start(out=outx_d.ap(), in_=xr)

            # ---- load constants ----
            sel2 = const.tile([P, 64], dt.bfloat16)
            selneg = const.tile([P, 16], dt.bfloat16)
            wpos8 = const.tile([BPC, P], dt.bfloat16)
            sel2w = const.tile([P, P], dt.bfloat16)
            nc.sync.dma_start(out=sel2w, in_=sel2w_d.ap())
            nc.sync.dma_start(out=sel2, in_=sel2_d.ap())
            nc.sync.dma_start(out=selneg, in_=selneg_d.ap())
            nc.sync.dma_start(out=wpos8, in_=wpos8_d.ap())

            # ---- phase 1: MT[g] = (T2 slice)^T @ x^T, in bf16 ----
            xT_f = const.tile([P, 8, N], dt.float32)
            tg_f = const.tile([P, 8, BPC * C], dt.float32)
            xT_b = const.tile([P, 8, N], dt.bfloat16)
            tg_b = const.tile([P, 8, BPC * C], dt.bfloat16)
            xT_view = xT_d.ap().rearrange("(kt p) n -> p kt n", p=P)
            tg_view = t2g_d.ap().rearrange("(kt p) m -> p kt m", p=P)
            for kt in range(8):
                nc.sync.dma_start(out=xT_f[:, kt, :], in_=xT_view[:, kt, :])
                nc.sync.dma_start(out=tg_f[:, kt, :], in_=tg_view[:, kt, :])
                # prologue casts ride the otherwise-idle ScalarE/GpSimd so the
                # VectorE stays free and the casts overlap the input DMAs
                nc.gpsimd.tensor_copy(xT_b[:, kt, :], xT_f[:, kt, :])
                nc.gpsimd.tensor_copy(tg_b[:, kt, :], tg_f[:, kt, :])

            MT = []
            for g in range(2):
                mt_ps = ps_mt.tile([P, N], dt.float32)
                for kt in range(8):
                    nc.tensor.matmul(
                        mt_ps,
                        lhsT=tg_b[:, kt, g * P:(g + 1) * P],
                        rhs=xT_b[:, kt, :],
                        start=(kt == 0),
                        stop=(kt == 7),
                    )
                mt_sb = const.tile([P, N], dt.bfloat16, tag=f"mt{g}")
                nc.vector.tensor_copy(mt_sb, mt_ps)
                # f32 copy OF THE bf16 value — scalar/bias APs must be f32;
                # exact upcast keeps the diagonal at exactly 0
                mt_f = const.tile([P, N], dt.float32, tag=f"mtf{g}")
                nc.scalar.copy(mt_f, mt_sb)
                # negated f32 copy of the bf16 value: ScalarE Relu bias needs
                # -m_i so that relu(1*m_j + (-m_i)) = relu(d), keeping every
                # slot on the same 2*relu(d)-d decomposition (diagonal stays
                # exactly 0 because both operands are the same bf16 value)
                mt_nf = const.tile([P, N], dt.float32, tag=f"mtnf{g}")
                nc.scalar.mul(mt_nf, mt_sb, -1.0)
                MT.append((mt_sb, mt_f, mt_nf))

            # ---- colsum path: csn[b, j] = -sum_c MT[(b,c), j]  (bf16-exact) ----
            cs_ps = ps_cs.tile([BPC, N], dt.float32)
            for g in range(2):
                nc.tensor.matmul(
                    cs_ps,
                    lhsT=selneg[:, g * 8:(g + 1) * 8],
                    rhs=MT[g][0],
                    start=(g == 0),
                    stop=(g == 1),
                )
            csn_b = const.tile([BPC, 2 * N], dt.bfloat16)
            nc.vector.tensor_copy(csn_b[:, :N], cs_ps)
            nc.vector.tensor_copy(csn_b[:, N:], cs_ps)
            # f32 copy OF THE bf16 value (so the exp bias matches the matmul
            # path bit-exactly on the diagonal)
            csn_f = const.tile([BPC, N], dt.float32)
            nc.vector.tensor_copy(csn_f, csn_b[:, :N])
            # gather to [(u b) = 128, blk = 16] via a DRAM bounce (transposed)
            cs_dram = dram.tile([N, BPC], dt.float32)
            nc.sync.dma_start(out=cs_dram[:].rearrange("i b -> b i"), in_=csn_f)
            csn_r = const.tile([P, NBLK], dt.float32)
            nc.sync.dma_start(
                out=csn_r,
                in_=cs_dram[:].rearrange("(blk u) b -> (u b) blk", blk=NBLK),
            )

            # ---- phase 2 (two i-blocks share each PSUM bank / matmul) ----
            acc = const.tile([P, NBLK], dt.float32)

            import contextlib
            loop_cm = tc.For_i(0, REPEAT, 1) if REPEAT > 1 else contextlib.nullcontext()
            with loop_cm:
              for bp in range(NBLK // 2):
                  l1 = ps_l1.tile([P, 2 * N], dt.float32)
                  for s in range(4):
                      for t in range(4):
                          u = 4 * t + s
                          for g in range(2):
                              a_t = apool.tile([P, 2 * N], dt.bfloat16, tag="a")
                              src, src_f, src_nf = MT[g]
                              on_act = _act_assign(s, t, g)
                              on_gps = _gps_assign(s, t, g)
                              for h in range(2):
                                  i = BLK * (2 * bp + h) + u
                                  dst = a_t[:, h * N:(h + 1) * N]
                                  if on_act:
                                      # relu(m_j - m_i) on the ScalarE: Relu
                                      # func with bias -m_i (negated f32 copy
                                      # of the same bf16 value, so the
                                      # diagonal is exactly 0)
                                      nc.scalar.activation(
                                          out=dst, in_=src, func=F.Relu,
                                          bias=src_nf[:, i:i + 1], scale=1.0,
                                      )
                                  elif on_gps:
                                      nc.gpsimd.tensor_scalar(
                                          dst, src, src_f[:, i:i + 1], 0.0,
                                          A.subtract, A.max,
                                      )
                                  else:
                                      # relu(m_j - m_i)
                                      nc.vector.tensor_scalar(
                                          dst, src, src_f[:, i:i + 1], 0.0,
                                          A.subtract, A.max,
                                      )
                              sel = sel2
                              w = 8 * s + 4 * g
                              if s == 0 and t == 0 and g == 0:
                                  # first MM of the pair: full-width selector
                                  # (zero-padded) opens one accumulation group
                                  # covering the whole [128, 512] region
                                  nc.tensor.matmul(
                                      l1, lhsT=sel2w, rhs=a_t,
                                      start=True, stop=False,
                                  )
                              else:
                                  nc.tensor.matmul(
                                      l1[32 * t:32 * t + 32, :],
                                      lhsT=sel[:, 32 - w:64 - w],
                                      rhs=a_t,
                                      start=False,
                                      stop=False,
                                      tile_position=(0, 32 * t),
                                  )
                  # add -colsum[b, j] to every row, close the accumulation group
                  nc.tensor.matmul(
                      l1, lhsT=wpos8, rhs=csn_b, start=False, stop=True,
                  )
                  for h in range(2):
                      blk = 2 * bp + h
                      e_t = epool.tile([P, N], dt.bfloat16, tag="e")
                      nc.scalar.activation(
                          out=e_t, in_=l1[:, h * N:(h + 1) * N], func=F.Exp,
                          bias=csn_r[:, blk:blk + 1], scale=-1.0,
                          accum_out=acc[:, blk:blk + 1],
                      )

            accm1 = const.tile([P, NBLK], dt.float32)
            nc.vector.tensor_scalar_sub(accm1, acc, 1.0)
            nc.sync.dma_start(
                out=outb_d.ap().rearrange("(blk u) b -> (u b) blk", blk=NBLK),
                in_=accm1,
            )

    nc.compile()
    return nc


def _consts():
    p = np.arange(P)
    sel2 = np.zeros((P, 64), np.float32)
    sel2[p, 32 + p // 32] = 2.0
    selneg = np.zeros((P, 16), np.float32)
    for g in range(2):
        selneg[p, 8 * g + 4 * g + p // 32] = -1.0
    m = np.arange(P)
    wpos8 = np.zeros((BPC, P), np.float32)
    wpos8[m % BPC, m] = 1.0
    sel2w = np.zeros((P, P), np.float32)
    sel2w[p, p // 32] = 2.0
    bf = ml_dtypes.bfloat16
    return (sel2.astype(bf), selneg.astype(bf),
            wpos8.astype(bf), sel2w.astype(bf))


def make_in_maps(x, T):
    x = np.asarray(x, dtype=np.float32)
    T = np.asarray(T, dtype=np.float32)
    sel2, selneg, wpos8, sel2w = _consts()
    xT = np.ascontiguousarray(x.T)
    T4 = T.reshape(A_DIM, B, C)
    rpc = N // NCORES
    in_maps = []
    for k in range(NCORES):
        t2g = np.ascontiguousarray(
            T4[:, k * BPC:(k + 1) * BPC, :].reshape(A_DIM, BPC * C))
        in_maps.append({
            "xT": xT,
            "t2g": t2g,
            "xrows": np.ascontiguousarray(x[k * rpc:(k + 1) * rpc]),
            "sel2": sel2, "selneg": selneg, "wpos8": wpos8,
            "sel2w": sel2w,
        })
    return in_maps


def assemble(results, x):
    full = np.empty((N, A_DIM + B), np.float32)
    rpc = N // NCORES
    for k in range(NCORES):
        full[k * rpc:(k + 1) * rpc, :A_DIM] = results[k]["out_x"]
        full[:, A_DIM + k * BPC:A_DIM + (k + 1) * BPC] = results[k]["out_b"]
    return full


def kernel(x, T):
    from concourse.bass_utils import run_bass_kernel_spmd

    if "nc" not in _cache:
        _cache["nc"] = build()
    nc = _cache["nc"]
    in_maps = make_in_maps(x, T)
    # plain execute path: never try to NTFF-trace inside the grading call
    prev = os.environ.get("BASS_NEVER_TRACE")
    os.environ["BASS_NEVER_TRACE"] = "1"
    try:
        res = run_bass_kernel_spmd(nc, in_maps, core_ids=list(range(NCORES)))
    finally:
        if prev is None:
            os.environ.pop("BASS_NEVER_TRACE", None)
        else:
            os.environ["BASS_NEVER_TRACE"] = prev
    return assemble(res.results, x)



# revision 29
# speedup vs baseline: 1.7769x; 1.7769x over previous
"""Trainium2 Bass kernel for nn_MinibatchDiscrimination1d.

  x [256,1024] f32, T [1024,64,32] f32
  M = (x @ T.reshape(1024, 2048)).reshape(256, 64, 32)
  l1[i,j,b] = sum_c |M[i,b,c] - M[j,b,c]|
  out = concat([x, sum_j exp(-l1) - 1], axis=1)   # [256, 1088]

Sharding: B=64 split across 8 cores (8 b's per core); x rows copied
through row-sharded.

Design:
  * Relu decomposition |d| = 2*relu(d) - d, so that every engine can
    produce its slot with ONE fused op (walrus rejects abs_max/bitwise
    ALU ops in TENSOR_SCALAR): DVE/Pool tensor_scalar (subtract m_i, max
    0), ScalarE Relu activation with bias -m_i.  Then
      l1[i,j] = 2*sum_c relu(d) - cs_j + cs_i,   cs_i = sum_c M[i,(b,c)]
    Both correction terms are accumulated INTO the l1 PSUM by two extra
    matmuls per block (a [8->128] broadcast of csn = -cs, and a rank-1
    K=1 matmul cs_i (x) ones), keeping exp bias-free so one Exp per PSUM
    group suffices.  The diagonal stays exactly 0 (same bf16 values, exact
    negation).
  * i<j symmetry: row-block I only computes columns j >= 16*I (F_I =
    256-16I).  The missing lower-triangle contribution for output row j
    comes from a per-block column-sum of the exp tile (PE ones-matmul into
    a shared [8, 240] PSUM accumulator).
  * Slots produced on ScalarE/GpSimd are written in fp8 (e4m3) as
    [128, 2, F] (both g groups) and reduced over c with a single
    full-width perf_mode=DoubleRow matmul (DoubleRow needs dst partition
    base 0); DVE slots stay bf16 (4x DVE mode) with two plain matmuls
    through a sliding 2.0-one-hot selector.
  * Blocks are packed in (I, 15-I) pairs: every PSUM group is exactly 272
    columns -> uniform engine load per group, one Exp activation per group.
    Group k's Exp/rowsum/colsum ops are emitted after group k+1's slot ops
    (software pipelining, avoids head-of-line blocking).
  * Row sums [128, 16] and column sums [8, 240] are DMA'd out contiguously;
    the final transpose/add/-1 happens on the host during unshard (avoids
    per-element-descriptor transpose DMAs on device).

Per-core layout: MT[g] = [128 partitions = (4 b x 32 c), 256 = rows], the
l1 PSUM tiles are [128 = (16 u x 8 b), 272].
"""

import os
import numpy as np
import ml_dtypes

N = 256
A_DIM = 1024
B = 64
C = 32
NCORES = 8
BPC = B // NCORES          # 8 b's per core
P = 128
NBLK = 16                  # 16 i-blocks of 16 rows
BLK = 16

# PSUM packing groups: blocks sharing one l1 tile / one Exp activation.
# Pairing block I with 15-I makes every group exactly 272 columns wide.
GROUPS = [(i, NBLK - 1 - i) for i in range(NBLK // 2)]

NA = int(os.environ.get("KERN_NA", "46"))   # i-slots on ScalarE (Relu)
NP = int(os.environ.get("KERN_NP", "50"))   # i-slots on GpSimd
TA = int(os.environ.get("KERN_TA", "0"))    # ScalarE tilt toward late groups
TP = int(os.environ.get("KERN_TP", "0"))    # GpSimd tilt toward late groups
A_BUFS = int(os.environ.get("KERN_A_BUFS", "20"))
A8_BUFS = int(os.environ.get("KERN_A8_BUFS", "16"))
L1_BUFS = int(os.environ.get("KERN_L1_BUFS", "3"))

_cache = {}


def _fi(i_blk):
    return 256 - BLK * i_blk


def _assign():
    """Per (block, u) engine: 'd' (DVE), 'a' (ScalarE), 'p' (GpSimd).
    Greedy LPT makespan assignment using the TimelineSim per-op cost model:
      DVE slot  = 2*(0.2604*F + 60.4) ns   (4x tensor_scalar)
      ACT slot  = 2*(0.8333*F + 185) ns    (Relu activation)
      Pool slot = 2*(1.389*F + 95) ns      (Q7 tensor_scalar)
    Each engine starts preloaded with its fixed non-slot work.  GpSimd is
    relatively cheapest on small-F slots (95ns fixed vs ACT's 185), so the
    greedy naturally sends small blocks to Pool and big ones to DVE.
    ScalarE/GpSimd slots are then spread across each block so the in-order
    PSUM matmul chain never waits on a burst of slow-engine producers."""
    fis = [_fi(I) for I in range(NBLK)]
    # fixed extras (ns)
    load = {
        'd': 16 * 60.4 + sum(0.52 * f for f in fis) + 392 + 327 + 330 + 76,
        'a': 8 * (0.8333 * 272 + 185) + 2 * 400 + 400 + 343 + 1300,
        'p': 2 * 1000.0,
    }
    eff = dict(load)
    slots = [(I, u) for I in range(NBLK) for u in range(BLK)]
    slots.sort(key=lambda s: -fis[s[0]])
    cost = {
        'd': lambda F: 2 * (0.2604 * F + 60.4),
        'a': lambda F: 2 * (0.8333 * F + 185),
        'p': lambda F: 2 * (1.389 * F + 95),
    }
    force = os.environ.get("KERN_FORCE_ENG")   # 'd'/'a'/'p': debug override
    chosen = {}
    for I, u in slots:
        F = fis[I]
        if force:
            e = force
        else:
            e = min(('d', 'a', 'p'), key=lambda e_: eff[e_] + cost[e_](F))
        chosen[(I, u)] = e
        eff[e] += cost[e](F)
    # u==0 must be DVE: its bf16 full-width matmul opens the PSUM region
    for I in range(NBLK):
        if chosen[(I, 0)] != 'd':
            swap = next((u for u in range(1, BLK) if chosen[(I, u)] == 'd'), None)
            if swap is not None:
                chosen[(I, swap)] = chosen[(I, 0)]
            chosen[(I, 0)] = 'd'
    # redistribute within each block: spread 'a'/'p' slots evenly over u
    table = {}
    for I in range(NBLK):
        engs = [chosen[(I, u)] for u in range(BLK)]
        na, npp = engs.count('a'), engs.count('p')
        order = [(u0 * 7 + I) % BLK for u0 in range(BLK)]
        seen = []
        for u in order:
            if u not in seen:
                seen.append(u)
        acts = set(seen[:na])
        pools = set(seen[na:na + npp])
        for u in range(BLK):
            table[(I, u)] = 'a' if u in acts else ('p' if u in pools else 'd')
    return table


def build():
    import concourse.bacc as bacc
    import concourse.tile as tile
    from concourse import mybir

    dt = mybir.dt
    A = mybir.AluOpType
    F = mybir.ActivationFunctionType

    assign = _assign()

    nc = bacc.Bacc("TRN2", target_bir_lowering=False, debug=False)

    # xT8/t2g8 are host-prearranged to [p, kt, n] so every DMA row is one
    # long contiguous run (cheap descriptors)
    xT8_d = nc.dram_tensor("xT8", [P, 8 * N], dt.float8e4, kind="ExternalInput")
    t2g8_d = nc.dram_tensor("t2g8", [P, 8 * BPC * C], dt.float8e4, kind="ExternalInput")
    xrows_d = nc.dram_tensor("xrows", [N // NCORES, A_DIM], dt.float32, kind="ExternalInput")
    selwb_d = nc.dram_tensor("selwb", [P, 2 * P], dt.bfloat16, kind="ExternalInput")
    w8f_d = nc.dram_tensor("w8f", [P, NBLK * 2 * P], dt.float8e4, kind="ExternalInput")
    onesb_d = nc.dram_tensor("onesb", [P, BPC], dt.bfloat16, kind="ExternalInput")
    selneg_d = nc.dram_tensor("selneg", [P, 2 * BPC], dt.bfloat16, kind="ExternalInput")
    wpos8_d = nc.dram_tensor("wpos8", [BPC, P], dt.bfloat16, kind="ExternalInput")
    onesrow_d = nc.dram_tensor("onesrow", [1, N], dt.bfloat16, kind="ExternalInput")

    DBG = os.environ.get("KERN_DEBUG")
    if DBG:
        dbg_csn = nc.dram_tensor("dbg_csn", [BPC, N], dt.float32, kind="ExternalOutput")
        dbg_csp = nc.dram_tensor("dbg_csp", [1, BPC * N], dt.float32, kind="ExternalOutput")
        dbg_l1 = nc.dram_tensor("dbg_l1", [P, 512], dt.float32, kind="ExternalOutput")
        dbg_et = nc.dram_tensor("dbg_et", [P, 512], dt.float32, kind="ExternalOutput")
    outacc_d = nc.dram_tensor("out_acc", [P, NBLK], dt.float32, kind="ExternalOutput")
    outcs_d = nc.dram_tensor("out_cs", [BPC, N - BLK], dt.float32, kind="ExternalOutput")
    outx_d = nc.dram_tensor("out_x", [N // NCORES, A_DIM], dt.float32, kind="ExternalOutput")

    with tile.TileContext(nc) as tc:
        with (
            tc.tile_pool(name="const", bufs=1) as const,
            tc.tile_pool(name="apool", bufs=A_BUFS) as apool,
            tc.tile_pool(name="a8pool", bufs=A8_BUFS) as a8pool,
            tc.tile_pool(name="epool", bufs=2) as epool,
            tc.tile_pool(name="ps_mt", bufs=2, space="PSUM") as ps_mt,
            tc.tile_pool(name="ps_l1", bufs=L1_BUFS, space="PSUM") as ps_l1,
            tc.tile_pool(name="ps_cs", bufs=1, space="PSUM") as ps_cs,
            tc.tile_pool(name="dram", bufs=1, space="DRAM") as dram,
        ):
            # ---- input DMAs: heavy inputs on SP/ACT queues, constants on
            # the (startup-idle) GpSimd SWDGE queue ----
            xT8 = const.tile([P, 8, N], dt.float8e4)
            tg8 = const.tile([P, 8, BPC * C], dt.float8e4)
            selwb = const.tile([P, 2 * P], dt.bfloat16)
            w8f = const.tile([P, NBLK, 2, P], dt.float8e4)
            onesb = const.tile([P, BPC], dt.bfloat16)
            selneg = const.tile([P, 2 * BPC], dt.bfloat16)
            wpos8 = const.tile([BPC, P], dt.bfloat16)
            ones_row = const.tile([1, N], dt.bfloat16)
            xr = const.tile([N // NCORES, A_DIM], dt.float32)
            tg_view = t2g8_d.ap().rearrange("p (kt m) -> p kt m", kt=8)
            xT_view = xT8_d.ap().rearrange("p (kt n) -> p kt n", kt=8)
            # dummy first ScalarE op: walrus inserts the ACT table load
            # before it, so the ~1.3us load overlaps the input DMAs instead
            # of blocking the first real activation
            dumb = const.tile([1, 2], dt.float32)
            nc.gpsimd.memset(dumb[:, :1], 0.0)
            nc.scalar.mul(dumb[:, 1:], dumb[:, :1], 1.0)
            # heavy inputs split across the SP and ACT HWDGE queues (halved
            # so the first phase-1 matmuls start early); late-needed
            # constants go through the GpSimd SWDGE queue (idle at startup)
            nc.sync.dma_start(out=tg8[:, :4], in_=tg_view[:, :4])
            nc.scalar.dma_start(out=xT8[:, :4], in_=xT_view[:, :4])
            nc.sync.dma_start(out=tg8[:, 4:], in_=tg_view[:, 4:])
            nc.scalar.dma_start(out=xT8[:, 4:], in_=xT_view[:, 4:])
            nc.scalar.dma_start(out=selwb, in_=selwb_d.ap())
            nc.gpsimd.dma_start(out=selneg, in_=selneg_d.ap())
            nc.gpsimd.dma_start(out=w8f, in_=w8f_d.ap().rearrange("p (u k m) -> p u k m", u=NBLK, k=2))
            nc.sync.dma_start(out=xr, in_=xrows_d.ap())
            nc.sync.dma_start(out=wpos8, in_=wpos8_d.ap())
            nc.sync.dma_start(out=onesb, in_=onesb_d.ap())
            nc.sync.dma_start(out=ones_row, in_=onesrow_d.ap())
            nc.sync.dma_start(out=outx_d.ap(), in_=xr)

            # ---- phase 1: MT[g] = (T2 slice)^T @ x^T, fp8 DoubleRow ----
            MT = []
            for g in range(2):
                mt_ps = ps_mt.tile([P, N], dt.float32)
                for pr in range(4):
                    nc.tensor.matmul(
                        mt_ps,
                        lhsT=tg8[:, 2 * pr:2 * pr + 2, g * P:(g + 1) * P],
                        rhs=xT8[:, 2 * pr:2 * pr + 2, :],
                        start=(pr == 0),
                        stop=(pr == 3),
                        perf_mode=mybir.MatmulPerfMode.DoubleRow,
                    )
                # bf16 working copy + negated f32 copy OF THE bf16 value so
                # the diagonal relu(m_i - m_i) is exactly 0 on every engine
                # (DVE/Pool use (add -m_i, max 0), ScalarE Relu bias -m_i)
                mt_sb = const.tile([P, N], dt.bfloat16, tag=f"mt{g}")
                mt_nf = const.tile([P, N], dt.float32, tag=f"mtnf{g}")
                if g == 0:
                    nc.vector.tensor_copy(mt_sb, mt_ps)
                    nc.scalar.mul(mt_nf, mt_sb, -1.0)
                else:
                    nc.scalar.copy(mt_sb, mt_ps)
                    nc.vector.tensor_scalar_mul(mt_nf, mt_sb, -1.0)
                # quarter-scaled copies for the fp8 path: device float8e4 is
                # IEEE-style e4m3 (inf at exponent 15, max finite 240), and
                # relu(d) can reach ~260.  0.25x is exact in bf16; the fp8
                # DoubleRow selector weights are 8.0 to compensate.
                mt_sq = const.tile([P, N], dt.bfloat16, tag=f"mtq{g}")
                nc.vector.tensor_scalar_mul(mt_sq, mt_sb, 0.25)
                mt_nq = const.tile([P, N], dt.float32, tag=f"mtnq{g}")
                nc.scalar.mul(mt_nq, mt_sb, -0.25)
                MT.append((mt_sb, mt_nf, mt_sq, mt_nq))

            # ---- colsum corrections: csn = -sum_c M  [8, 256] (bf16) ----
            csn_ps = ps_cs.tile([BPC, N], dt.float32, tag="csn")
            for g in range(2):
                nc.tensor.matmul(
                    csn_ps,
                    lhsT=selneg[:, g * BPC:(g + 1) * BPC],
                    rhs=MT[g][0],
                    start=(g == 0),
                    stop=(g == 1),
                )
            csn_b = const.tile([BPC, N], dt.bfloat16)
            nc.vector.tensor_copy(csn_b, csn_ps)
            # csp = +cs in bf16 (exact negation of the same bf16 values)
            csp_b = const.tile([BPC, N], dt.bfloat16)
            nc.scalar.mul(csp_b, csn_ps, -1.0)
            # bounce csp through DRAM into [1, (i b)] so a K=1 rank-1 matmul
            # can broadcast cs_i along j into each block's PSUM rows
            cs_dram = dram.tile([1, BPC * N], dt.bfloat16)
            nc.sync.dma_start(
                out=cs_dram[:].rearrange("o (i b) -> (o b) i", b=BPC), in_=csp_b)
            cspT = const.tile([1, BPC * N], dt.bfloat16)
            nc.sync.dma_start(out=cspT, in_=cs_dram[:])
            if DBG:
                csnc = const.tile([BPC, N], dt.float32, tag="dbgcsn")
                nc.vector.tensor_copy(csnc, csn_b)
                nc.sync.dma_start(out=dbg_csn.ap(), in_=csnc)
                cspc = const.tile([1, BPC * N], dt.float32, tag="dbgcsp")
                nc.vector.tensor_copy(cspc, cspT)
                nc.sync.dma_start(out=dbg_csp.ap(), in_=cspc)

            # ---- phase 2 ----
            acc = const.tile([P, NBLK], dt.float32)
            cs_ps = ps_cs.tile([BPC, N - BLK], dt.float32, tag="cse")

            def emit_slots(grp):
                wtot = sum(_fi(I) for I in grp)
                l1 = ps_l1.tile([P, 512], dt.float32, tag="l1")
                c0 = 0
                for I in grp:
                    Fw = _fi(I)
                    jlo = BLK * I
                    for u in range(BLK):
                        i = BLK * I + u
                        eng = assign[(I, u)]
                        if eng == 'd':
                            a_t = apool.tile([P, 512], dt.bfloat16, tag="a")
                            for g in range(2):
                                src, src_nf = MT[g][:2]
                                nc.vector.tensor_scalar(
                                    a_t[:, g * Fw:(g + 1) * Fw],
                                    src[:, jlo:jlo + Fw],
                                    src_nf[:, i:i + 1], 0.0,
                                    A.add, A.max,
                                )
                            for g in range(2):
                                s = P - 8 * u - 4 * g
                                nc.tensor.matmul(
                                    l1[:, c0:c0 + Fw],
                                    lhsT=selwb[:, s:s + P],
                                    rhs=a_t[:, g * Fw:(g + 1) * Fw],
                                    start=(u == 0 and g == 0),
                                    stop=False,
                                )
                        else:
                            a8 = a8pool.tile([P, 2, N], dt.float8e4, tag="a8")
                            for g in range(2):
                                src_q, src_nq = MT[g][2:]
                                if eng == 'a':
                                    nc.scalar.activation(
                                        out=a8[:, g, :Fw],
                                        in_=src_q[:, jlo:jlo + Fw],
                                        func=F.Relu,
                                        bias=src_nq[:, i:i + 1], scale=1.0,
                                    )
                                else:
                                    nc.gpsimd.tensor_scalar(
                                        a8[:, g, :Fw],
                                        src_q[:, jlo:jlo + Fw],
                                        src_nq[:, i:i + 1], 0.0,
                                        A.add, A.max,
                                    )
                            nc.tensor.matmul(
                                l1[:, c0:c0 + Fw],
                                lhsT=w8f[:, u],
                                rhs=a8[:, :, :Fw],
                                start=(u == 0),
                                stop=False,
                                perf_mode=mybir.MatmulPerfMode.DoubleRow,
                            )
                    # corrections: -cs_j broadcast down columns, +cs_i
                    # along rows (rank-1 closes the block's accumulation)
                    nc.tensor.matmul(
                        l1[:, c0:c0 + Fw],
                        lhsT=wpos8,
                        rhs=csn_b[:, jlo:jlo + Fw],
                        start=False, stop=False,
                    )
                    nc.tensor.matmul(
                        l1[:, c0:c0 + Fw],
                        lhsT=cspT[0:1, BPC * jlo:BPC * (jlo + BLK)],
                        rhs=ones_row[:, :Fw],
                        start=False, stop=True,
                    )
                    c0 += Fw
                return l1, wtot

            def emit_wrapup(grp, l1, wtot):
                # one Exp for the whole packed group
                e_t = epool.tile([P, 512], dt.bfloat16, tag="e")
                if DBG and grp[0] == 0:
                    l1c = const.tile([P, 512], dt.float32, tag="dbgl1")
                    nc.vector.tensor_copy(l1c, l1)
                    nc.sync.dma_start(out=dbg_l1.ap(), in_=l1c)
                nc.scalar.activation(
                    out=e_t[:, :wtot], in_=l1[:, :wtot], func=F.Exp, scale=-1.0,
                )
                if DBG and grp[0] == 0:
                    etc = const.tile([P, 512], dt.float32, tag="dbget")
                    nc.vector.tensor_copy(etc, e_t)
                    nc.sync.dma_start(out=dbg_et.ap(), in_=etc)
                # per block: row sums + column-sum matmul
                c0 = 0
                for I in grp:
                    Fw = _fi(I)
                    nc.vector.tensor_reduce(
                        out=acc[:, I:I + 1],
                        in_=e_t[:, c0:c0 + Fw],
                        axis=mybir.AxisListType.X,
                        op=A.add,
                    )
                    if I < NBLK - 1:
                        # cs[b, j-16] += sum_u e_t[(u, b), j] for j > block I
                        # start: block 0's matmul covers the full [8, 240]
                        # region and is emitted first; stop: last emitted
                        # colsum (PE executes matmuls in program order)
                        nc.tensor.matmul(
                            cs_ps[:, BLK * I:],
                            lhsT=onesb,
                            rhs=e_t[:, c0 + BLK:c0 + Fw],
                            start=(I == 0),
                            stop=(I == GROUPS[-1][-1]),
                        )
                    c0 += Fw

            # software pipeline: group k's Exp/sums are emitted after group
            # k+1's slot ops so they never head-of-line-block the engines
            pending = None
            for grp in GROUPS:
                l1, wtot = emit_slots(grp)
                if pending is not None:
                    emit_wrapup(*pending)
                pending = (grp, l1, wtot)
            emit_wrapup(*pending)

            # ---- outputs: contiguous, transpose/add/-1 happen on host ----
            cs_sb = const.tile([BPC, N - BLK], dt.float32)
            nc.scalar.copy(cs_sb, cs_ps)
            nc.sync.dma_start(out=outcs_d.ap(), in_=cs_sb)
            nc.scalar.dma_start(out=outacc_d.ap(), in_=acc)

    nc.compile()
    return nc


def _consts():
    p = np.arange(P)
    bl = p // 32                       # b_local of partition (b_l, c)
    selwb = np.zeros((P, 2 * P), np.float32)
    selwb[p, P + bl] = 2.0             # slice [s:s+128], s=128-8u-4g -> col 8u+4g+b_l
    w8f = np.zeros((P, NBLK, 2, P), np.float32)
    for u in range(NBLK):
        for k in range(2):
            w8f[p, u, k, 8 * u + 4 * k + bl] = 8.0
    onesb = np.zeros((P, BPC), np.float32)
    onesb[p, p % BPC] = 1.0
    selneg = np.zeros((P, 2 * BPC), np.float32)
    for g in range(2):
        selneg[p, BPC * g + 4 * g + bl] = -1.0
    wpos8 = np.zeros((BPC, P), np.float32)
    m = np.arange(P)
    wpos8[m % BPC, m] = 1.0
    onesrow = np.ones((1, N), np.float32)
    f8 = ml_dtypes.float8_e4m3fn
    bf = ml_dtypes.bfloat16
    return (selwb.astype(bf), w8f.reshape(P, -1).astype(f8), onesb.astype(bf),
            selneg.astype(bf), wpos8.astype(bf), onesrow.astype(bf))


def make_in_maps(x, T):
    x = np.asarray(x, dtype=np.float32)
    T = np.asarray(T, dtype=np.float32)
    selwb, w8f, onesb, selneg, wpos8, onesrow = _consts()
    f8 = ml_dtypes.float8_e4m3fn
    # [p, kt*n] prearranged layout: row p holds x^T[kt*128 + p, :] for kt=0..7
    xT = np.ascontiguousarray(x.T)                      # [1024, 256]
    xT8 = np.ascontiguousarray(
        xT.reshape(8, P, N).transpose(1, 0, 2).reshape(P, 8 * N)).astype(f8)
    T4 = T.reshape(A_DIM, B, C)
    rpc = N // NCORES
    in_maps = []
    for k in range(NCORES):
        t2g = T4[:, k * BPC:(k + 1) * BPC, :].reshape(A_DIM, BPC * C)
        t2g8 = np.ascontiguousarray(
            t2g.reshape(8, P, BPC * C).transpose(1, 0, 2).reshape(P, -1)).astype(f8)
        in_maps.append({
            "xT8": xT8,
            "t2g8": t2g8,
            "xrows": np.ascontiguousarray(x[k * rpc:(k + 1) * rpc]),
            "selwb": selwb, "w8f": w8f, "onesb": onesb,
            "selneg": selneg, "wpos8": wpos8, "onesrow": onesrow,
        })
    return in_maps


def assemble(results, x):
    full = np.empty((N, A_DIM + B), np.float32)
    rpc = N // NCORES
    for k in range(NCORES):
        full[k * rpc:(k + 1) * rpc, :A_DIM] = results[k]["out_x"]
        acc = np.asarray(results[k]["out_acc"], np.float32)   # [(u b), blk]
        cs = np.asarray(results[k]["out_cs"], np.float32)     # [b, j-16]
        blk = acc.reshape(BLK, BPC, NBLK).transpose(2, 0, 1).reshape(N, BPC) - 1.0
        blk[BLK:, :] += cs.T
        full[:, A_DIM + k * BPC:A_DIM + (k + 1) * BPC] = blk
    return full


def kernel(x, T):
    from concourse.bass_utils import run_bass_kernel_spmd

    if "nc" not in _cache:
        _cache["nc"] = build()
    nc = _cache["nc"]
    in_maps = make_in_maps(x, T)
    prev = os.environ.get("BASS_NEVER_TRACE")
    os.environ["BASS_NEVER_TRACE"] = "1"
    try:
        res = run_bass_kernel_spmd(nc, in_maps, core_ids=list(range(NCORES)))
    finally:
        if prev is None:
            os.environ.pop("BASS_NEVER_TRACE", None)
        else:
            os.environ["BASS_NEVER_TRACE"] = prev
    return assemble(res.results, x)


# revision 30
# speedup vs baseline: 1.7884x; 1.0064x over previous
"""Trainium2 Bass kernel for nn_MinibatchDiscrimination1d.

  x [256,1024] f32, T [1024,64,32] f32
  M = (x @ T.reshape(1024, 2048)).reshape(256, 64, 32)
  l1[i,j,b] = sum_c |M[i,b,c] - M[j,b,c]|
  out = concat([x, sum_j exp(-l1) - 1], axis=1)   # [256, 1088]

Sharding: B=64 split across 8 cores (8 b's per core); x rows copied
through row-sharded.

Design:
  * Relu decomposition |d| = 2*relu(d) - d, so that every engine can
    produce its slot with ONE fused op (walrus rejects abs_max/bitwise
    ALU ops in TENSOR_SCALAR): DVE/Pool tensor_scalar (subtract m_i, max
    0), ScalarE Relu activation with bias -m_i.  Then
      l1[i,j] = 2*sum_c relu(d) - cs_j + cs_i,   cs_i = sum_c M[i,(b,c)]
    Both correction terms are accumulated INTO the l1 PSUM by two extra
    matmuls per block (a [8->128] broadcast of csn = -cs, and a rank-1
    K=1 matmul cs_i (x) ones), keeping exp bias-free so one Exp per PSUM
    group suffices.  The diagonal stays exactly 0 (same bf16 values, exact
    negation).
  * i<j symmetry: row-block I only computes columns j >= 16*I (F_I =
    256-16I).  The missing lower-triangle contribution for output row j
    comes from a per-block column-sum of the exp tile (PE ones-matmul into
    a shared [8, 240] PSUM accumulator).
  * Slots produced on ScalarE/GpSimd are written in fp8 (e4m3) as
    [128, 2, F] (both g groups) and reduced over c with a single
    full-width perf_mode=DoubleRow matmul (DoubleRow needs dst partition
    base 0); DVE slots stay bf16 (4x DVE mode) with two plain matmuls
    through a sliding 2.0-one-hot selector.
  * Blocks are packed in (I, 15-I) pairs: every PSUM group is exactly 272
    columns -> uniform engine load per group, one Exp activation per group.
    Group k's Exp/rowsum/colsum ops are emitted after group k+1's slot ops
    (software pipelining, avoids head-of-line blocking).
  * Row sums [128, 16] and column sums [8, 240] are DMA'd out contiguously;
    the final transpose/add/-1 happens on the host during unshard (avoids
    per-element-descriptor transpose DMAs on device).

Per-core layout: MT[g] = [128 partitions = (4 b x 32 c), 256 = rows], the
l1 PSUM tiles are [128 = (16 u x 8 b), 272].
"""

import os
import numpy as np
import ml_dtypes

N = 256
A_DIM = 1024
B = 64
C = 32
NCORES = 8
BPC = B // NCORES          # 8 b's per core
P = 128
NBLK = 16                  # 16 i-blocks of 16 rows
BLK = 16

# PSUM packing groups: blocks sharing one l1 tile / one Exp activation.
# Pairing block I with 15-I makes every group exactly 272 columns wide.
GROUPS = [(i, NBLK - 1 - i) for i in range(NBLK // 2)]

NA = int(os.environ.get("KERN_NA", "46"))   # i-slots on ScalarE (Relu)
NP = int(os.environ.get("KERN_NP", "50"))   # i-slots on GpSimd
TA = int(os.environ.get("KERN_TA", "0"))    # ScalarE tilt toward late groups
TP = int(os.environ.get("KERN_TP", "0"))    # GpSimd tilt toward late groups
A_BUFS = int(os.environ.get("KERN_A_BUFS", "28"))
A8_BUFS = int(os.environ.get("KERN_A8_BUFS", "20"))
L1_BUFS = int(os.environ.get("KERN_L1_BUFS", "3"))

_cache = {}


def _fi(i_blk):
    return 256 - BLK * i_blk


def _assign():
    """Per (block, u) engine: 'd' (DVE), 'a' (ScalarE), 'p' (GpSimd).
    Greedy LPT makespan assignment using the TimelineSim per-op cost model:
      DVE slot  = 2*(0.2604*F + 60.4) ns   (4x tensor_scalar)
      ACT slot  = 2*(0.8333*F + 185) ns    (Relu activation)
      Pool slot = 2*(1.389*F + 95) ns      (Q7 tensor_scalar)
    Each engine starts preloaded with its fixed non-slot work.  GpSimd is
    relatively cheapest on small-F slots (95ns fixed vs ACT's 185), so the
    greedy naturally sends small blocks to Pool and big ones to DVE.
    ScalarE/GpSimd slots are then spread across each block so the in-order
    PSUM matmul chain never waits on a burst of slow-engine producers."""
    fis = [_fi(I) for I in range(NBLK)]
    # fixed extras (ns)
    load = {
        'd': 16 * 60.4 + sum(0.52 * f for f in fis) + 392 + 327 + 330 + 76,
        'a': 8 * (0.8333 * 272 + 185) + 2 * 400 + 400 + 343 + 1300,
        'p': 2 * 1000.0,
    }
    eff = dict(load)
    slots = [(I, u) for I in range(NBLK) for u in range(BLK)]
    slots.sort(key=lambda s: -fis[s[0]])
    cost = {
        'd': lambda F: 2 * (0.2604 * F + 60.4),
        'a': lambda F: 2 * (0.8333 * F + 185),
        'p': lambda F: 2 * (1.389 * F + 95),
    }
    force = os.environ.get("KERN_FORCE_ENG")   # 'd'/'a'/'p': debug override
    chosen = {}
    for I, u in slots:
        F = fis[I]
        if force:
            e = force
        else:
            e = min(('d', 'a', 'p'), key=lambda e_: eff[e_] + cost[e_](F))
        chosen[(I, u)] = e
        eff[e] += cost[e](F)
    # u==0 must be DVE: its bf16 full-width matmul opens the PSUM region
    for I in range(NBLK):
        if chosen[(I, 0)] != 'd':
            swap = next((u for u in range(1, BLK) if chosen[(I, u)] == 'd'), None)
            if swap is not None:
                chosen[(I, swap)] = chosen[(I, 0)]
            chosen[(I, 0)] = 'd'
    # redistribute within each block: spread 'a'/'p' slots evenly over u
    table = {}
    for I in range(NBLK):
        engs = [chosen[(I, u)] for u in range(BLK)]
        na, npp = engs.count('a'), engs.count('p')
        order = [(u0 * 7 + I) % BLK for u0 in range(BLK)]
        seen = []
        for u in order:
            if u not in seen:
                seen.append(u)
        acts = set(seen[:na])
        pools = set(seen[na:na + npp])
        for u in range(BLK):
            table[(I, u)] = 'a' if u in acts else ('p' if u in pools else 'd')
    return table


def build():
    import concourse.bacc as bacc
    import concourse.tile as tile
    from concourse import mybir

    dt = mybir.dt
    A = mybir.AluOpType
    F = mybir.ActivationFunctionType

    assign = _assign()

    nc = bacc.Bacc("TRN2", target_bir_lowering=False, debug=False)

    # xT8/t2g8 are host-prearranged to [p, kt, n] so every DMA row is one
    # long contiguous run (cheap descriptors)
    xT8_d = nc.dram_tensor("xT8", [P, 8 * N], dt.float8e4, kind="ExternalInput")
    t2g8_d = nc.dram_tensor("t2g8", [P, 8 * BPC * C], dt.float8e4, kind="ExternalInput")
    xrows_d = nc.dram_tensor("xrows", [N // NCORES, A_DIM], dt.float32, kind="ExternalInput")
    selwb_d = nc.dram_tensor("selwb", [P, 2 * P], dt.bfloat16, kind="ExternalInput")
    w8f_d = nc.dram_tensor("w8f", [P, NBLK * 2 * P], dt.float8e4, kind="ExternalInput")
    onesb_d = nc.dram_tensor("onesb", [P, BPC], dt.bfloat16, kind="ExternalInput")
    selneg_d = nc.dram_tensor("selneg", [P, 2 * BPC], dt.bfloat16, kind="ExternalInput")
    wpos8_d = nc.dram_tensor("wpos8", [BPC, P], dt.bfloat16, kind="ExternalInput")
    onesrow_d = nc.dram_tensor("onesrow", [1, N], dt.bfloat16, kind="ExternalInput")

    DBG = os.environ.get("KERN_DEBUG")
    if DBG:
        dbg_csn = nc.dram_tensor("dbg_csn", [BPC, N], dt.float32, kind="ExternalOutput")
        dbg_csp = nc.dram_tensor("dbg_csp", [1, BPC * N], dt.float32, kind="ExternalOutput")
        dbg_l1 = nc.dram_tensor("dbg_l1", [P, 512], dt.float32, kind="ExternalOutput")
        dbg_et = nc.dram_tensor("dbg_et", [P, 512], dt.float32, kind="ExternalOutput")
    outacc_d = nc.dram_tensor("out_acc", [P, NBLK], dt.float32, kind="ExternalOutput")
    outcs_d = nc.dram_tensor("out_cs", [BPC, N - BLK], dt.float32, kind="ExternalOutput")
    outx_d = nc.dram_tensor("out_x", [N // NCORES, A_DIM], dt.float32, kind="ExternalOutput")

    with tile.TileContext(nc) as tc:
        with (
            tc.tile_pool(name="const", bufs=1) as const,
            tc.tile_pool(name="apool", bufs=A_BUFS) as apool,
            tc.tile_pool(name="a8pool", bufs=A8_BUFS) as a8pool,
            tc.tile_pool(name="epool", bufs=2) as epool,
            tc.tile_pool(name="ps_mt", bufs=2, space="PSUM") as ps_mt,
            tc.tile_pool(name="ps_l1", bufs=L1_BUFS, space="PSUM") as ps_l1,
            tc.tile_pool(name="ps_cs", bufs=1, space="PSUM") as ps_cs,
            tc.tile_pool(name="dram", bufs=1, space="DRAM") as dram,
        ):
            # ---- input DMAs: heavy inputs on SP/ACT queues, constants on
            # the (startup-idle) GpSimd SWDGE queue ----
            xT8 = const.tile([P, 8, N], dt.float8e4)
            tg8 = const.tile([P, 8, BPC * C], dt.float8e4)
            selwb = const.tile([P, 2 * P], dt.bfloat16)
            w8f = const.tile([P, NBLK, 2, P], dt.float8e4)
            onesb = const.tile([P, BPC], dt.bfloat16)
            selneg = const.tile([P, 2 * BPC], dt.bfloat16)
            wpos8 = const.tile([BPC, P], dt.bfloat16)
            ones_row = const.tile([1, N], dt.bfloat16)
            xr = const.tile([N // NCORES, A_DIM], dt.float32)
            tg_view = t2g8_d.ap().rearrange("p (kt m) -> p kt m", kt=8)
            xT_view = xT8_d.ap().rearrange("p (kt n) -> p kt n", kt=8)
            # dummy first ScalarE op: walrus inserts the ACT table load
            # before it, so the ~1.3us load overlaps the input DMAs instead
            # of blocking the first real activation
            dumb = const.tile([1, 2], dt.float32)
            nc.gpsimd.memset(dumb[:, :1], 0.0)
            nc.scalar.mul(dumb[:, 1:], dumb[:, :1], 1.0)
            # heavy inputs split across the SP and ACT HWDGE queues (halved
            # so the first phase-1 matmuls start early); late-needed
            # constants go through the GpSimd SWDGE queue (idle at startup)
            nc.sync.dma_start(out=tg8[:, :4], in_=tg_view[:, :4])
            nc.scalar.dma_start(out=xT8[:, :4], in_=xT_view[:, :4])
            nc.sync.dma_start(out=tg8[:, 4:], in_=tg_view[:, 4:])
            nc.scalar.dma_start(out=xT8[:, 4:], in_=xT_view[:, 4:])
            nc.scalar.dma_start(out=selwb, in_=selwb_d.ap())
            nc.gpsimd.dma_start(out=selneg, in_=selneg_d.ap())
            nc.gpsimd.dma_start(out=w8f, in_=w8f_d.ap().rearrange("p (u k m) -> p u k m", u=NBLK, k=2))
            nc.sync.dma_start(out=xr, in_=xrows_d.ap())
            nc.sync.dma_start(out=wpos8, in_=wpos8_d.ap())
            nc.sync.dma_start(out=onesb, in_=onesb_d.ap())
            nc.sync.dma_start(out=ones_row, in_=onesrow_d.ap())
            nc.sync.dma_start(out=outx_d.ap(), in_=xr)

            # ---- phase 1: MT[g] = (T2 slice)^T @ x^T, fp8 DoubleRow ----
            MT = []
            for g in range(2):
                mt_ps = ps_mt.tile([P, N], dt.float32)
                for pr in range(4):
                    nc.tensor.matmul(
                        mt_ps,
                        lhsT=tg8[:, 2 * pr:2 * pr + 2, g * P:(g + 1) * P],
                        rhs=xT8[:, 2 * pr:2 * pr + 2, :],
                        start=(pr == 0),
                        stop=(pr == 3),
                        perf_mode=mybir.MatmulPerfMode.DoubleRow,
                    )
                # bf16 working copy + negated f32 copy OF THE bf16 value so
                # the diagonal relu(m_i - m_i) is exactly 0 on every engine
                # (DVE/Pool use (add -m_i, max 0), ScalarE Relu bias -m_i)
                mt_sb = const.tile([P, N], dt.bfloat16, tag=f"mt{g}")
                mt_nf = const.tile([P, N], dt.float32, tag=f"mtnf{g}")
                if g == 0:
                    nc.vector.tensor_copy(mt_sb, mt_ps)
                    nc.scalar.mul(mt_nf, mt_sb, -1.0)
                else:
                    nc.scalar.copy(mt_sb, mt_ps)
                    nc.vector.tensor_scalar_mul(mt_nf, mt_sb, -1.0)
                # quarter-scaled copies for the fp8 path: device float8e4 is
                # IEEE-style e4m3 (inf at exponent 15, max finite 240), and
                # relu(d) can reach ~260.  0.25x is exact in bf16; the fp8
                # DoubleRow selector weights are 8.0 to compensate.
                mt_sq = const.tile([P, N], dt.bfloat16, tag=f"mtq{g}")
                nc.vector.tensor_scalar_mul(mt_sq, mt_sb, 0.25)
                mt_nq = const.tile([P, N], dt.float32, tag=f"mtnq{g}")
                nc.scalar.mul(mt_nq, mt_sb, -0.25)
                MT.append((mt_sb, mt_nf, mt_sq, mt_nq))

            # ---- colsum corrections: csn = -sum_c M  [8, 256] (bf16) ----
            csn_ps = ps_cs.tile([BPC, N], dt.float32, tag="csn")
            for g in range(2):
                nc.tensor.matmul(
                    csn_ps,
                    lhsT=selneg[:, g * BPC:(g + 1) * BPC],
                    rhs=MT[g][0],
                    start=(g == 0),
                    stop=(g == 1),
                )
            csn_b = const.tile([BPC, N], dt.bfloat16)
            nc.vector.tensor_copy(csn_b, csn_ps)
            # csp = +cs in bf16 (exact negation of the same bf16 values)
            csp_b = const.tile([BPC, N], dt.bfloat16)
            nc.scalar.mul(csp_b, csn_ps, -1.0)
            # bounce csp through DRAM into [1, (i b)] so a K=1 rank-1 matmul
            # can broadcast cs_i along j into each block's PSUM rows
            cs_dram = dram.tile([1, BPC * N], dt.bfloat16)
            nc.sync.dma_start(
                out=cs_dram[:].rearrange("o (i b) -> (o b) i", b=BPC), in_=csp_b)
            cspT = const.tile([1, BPC * N], dt.bfloat16)
            nc.sync.dma_start(out=cspT, in_=cs_dram[:])
            if DBG:
                csnc = const.tile([BPC, N], dt.float32, tag="dbgcsn")
                nc.vector.tensor_copy(csnc, csn_b)
                nc.sync.dma_start(out=dbg_csn.ap(), in_=csnc)
                cspc = const.tile([1, BPC * N], dt.float32, tag="dbgcsp")
                nc.vector.tensor_copy(cspc, cspT)
                nc.sync.dma_start(out=dbg_csp.ap(), in_=cspc)

            # ---- phase 2 ----
            acc = const.tile([P, NBLK], dt.float32)
            cs_ps = ps_cs.tile([BPC, N - BLK], dt.float32, tag="cse")

            def emit_slots(grp):
                wtot = sum(_fi(I) for I in grp)
                l1 = ps_l1.tile([P, 512], dt.float32, tag="l1")
                c0 = 0
                for I in grp:
                    Fw = _fi(I)
                    jlo = BLK * I
                    for u in range(BLK):
                        i = BLK * I + u
                        eng = assign[(I, u)]
                        if eng == 'd':
                            a_t = apool.tile([P, 512], dt.bfloat16, tag="a")
                            for g in range(2):
                                src, src_nf = MT[g][:2]
                                nc.vector.tensor_scalar(
                                    a_t[:, g * Fw:(g + 1) * Fw],
                                    src[:, jlo:jlo + Fw],
                                    src_nf[:, i:i + 1], 0.0,
                                    A.add, A.max,
                                )
                            for g in range(2):
                                s = P - 8 * u - 4 * g
                                nc.tensor.matmul(
                                    l1[:, c0:c0 + Fw],
                                    lhsT=selwb[:, s:s + P],
                                    rhs=a_t[:, g * Fw:(g + 1) * Fw],
                                    start=(u == 0 and g == 0),
                                    stop=False,
                                )
                        else:
                            a8 = a8pool.tile([P, 2, N], dt.float8e4, tag="a8")
                            for g in range(2):
                                src_q, src_nq = MT[g][2:]
                                if eng == 'a':
                                    nc.scalar.activation(
                                        out=a8[:, g, :Fw],
                                        in_=src_q[:, jlo:jlo + Fw],
                                        func=F.Relu,
                                        bias=src_nq[:, i:i + 1], scale=1.0,
                                    )
                                else:
                                    nc.gpsimd.tensor_scalar(
                                        a8[:, g, :Fw],
                                        src_q[:, jlo:jlo + Fw],
                                        src_nq[:, i:i + 1], 0.0,
                                        A.add, A.max,
                                    )
                            nc.tensor.matmul(
                                l1[:, c0:c0 + Fw],
                                lhsT=w8f[:, u],
                                rhs=a8[:, :, :Fw],
                                start=(u == 0),
                                stop=False,
                                perf_mode=mybir.MatmulPerfMode.DoubleRow,
                            )
                    # corrections: -cs_j broadcast down columns, +cs_i
                    # along rows (rank-1 closes the block's accumulation)
                    nc.tensor.matmul(
                        l1[:, c0:c0 + Fw],
                        lhsT=wpos8,
                        rhs=csn_b[:, jlo:jlo + Fw],
                        start=False, stop=False,
                    )
                    nc.tensor.matmul(
                        l1[:, c0:c0 + Fw],
                        lhsT=cspT[0:1, BPC * jlo:BPC * (jlo + BLK)],
                        rhs=ones_row[:, :Fw],
                        start=False, stop=True,
                    )
                    c0 += Fw
                return l1, wtot

            def emit_wrapup(grp, l1, wtot):
                # one Exp for the whole packed group
                e_t = epool.tile([P, 512], dt.bfloat16, tag="e")
                if DBG and grp[0] == 0:
                    l1c = const.tile([P, 512], dt.float32, tag="dbgl1")
                    nc.vector.tensor_copy(l1c, l1)
                    nc.sync.dma_start(out=dbg_l1.ap(), in_=l1c)
                nc.scalar.activation(
                    out=e_t[:, :wtot], in_=l1[:, :wtot], func=F.Exp, scale=-1.0,
                )
                if DBG and grp[0] == 0:
                    etc = const.tile([P, 512], dt.float32, tag="dbget")
                    nc.vector.tensor_copy(etc, e_t)
                    nc.sync.dma_start(out=dbg_et.ap(), in_=etc)
                # per block: row sums + column-sum matmul
                c0 = 0
                for I in grp:
                    Fw = _fi(I)
                    nc.vector.tensor_reduce(
                        out=acc[:, I:I + 1],
                        in_=e_t[:, c0:c0 + Fw],
                        axis=mybir.AxisListType.X,
                        op=A.add,
                    )
                    if I < NBLK - 1:
                        # cs[b, j-16] += sum_u e_t[(u, b), j] for j > block I
                        # start: block 0's matmul covers the full [8, 240]
                        # region and is emitted first; stop: last emitted
                        # colsum (PE executes matmuls in program order)
                        nc.tensor.matmul(
                            cs_ps[:, BLK * I:],
                            lhsT=onesb,
                            rhs=e_t[:, c0 + BLK:c0 + Fw],
                            start=(I == 0),
                            stop=(I == GROUPS[-1][-1]),
                        )
                    c0 += Fw

            # software pipeline: group k's Exp/sums are emitted after group
            # k+1's slot ops so they never head-of-line-block the engines
            pending = None
            for grp in GROUPS:
                l1, wtot = emit_slots(grp)
                if pending is not None:
                    emit_wrapup(*pending)
                pending = (grp, l1, wtot)
            emit_wrapup(*pending)

            # ---- outputs: contiguous, transpose/add/-1 happen on host ----
            cs_sb = const.tile([BPC, N - BLK], dt.float32)
            nc.scalar.copy(cs_sb, cs_ps)
            nc.sync.dma_start(out=outcs_d.ap(), in_=cs_sb)
            nc.scalar.dma_start(out=outacc_d.ap(), in_=acc)

    nc.compile()
    return nc


def _consts():
    p = np.arange(P)
    bl = p // 32                       # b_local of partition (b_l, c)
    selwb = np.zeros((P, 2 * P), np.float32)
    selwb[p, P + bl] = 2.0             # slice [s:s+128], s=128-8u-4g -> col 8u+4g+b_l
    w8f = np.zeros((P, NBLK, 2, P), np.float32)
    for u in range(NBLK):
        for k in range(2):
            w8f[p, u, k, 8 * u + 4 * k + bl] = 8.0
    onesb = np.zeros((P, BPC), np.float32)
    onesb[p, p % BPC] = 1.0
    selneg = np.zeros((P, 2 * BPC), np.float32)
    for g in range(2):
        selneg[p, BPC * g + 4 * g + bl] = -1.0
    wpos8 = np.zeros((BPC, P), np.float32)
    m = np.arange(P)
    wpos8[m % BPC, m] = 1.0
    onesrow = np.ones((1, N), np.float32)
    f8 = ml_dtypes.float8_e4m3fn
    bf = ml_dtypes.bfloat16
    return (selwb.astype(bf), w8f.reshape(P, -1).astype(f8), onesb.astype(bf),
            selneg.astype(bf), wpos8.astype(bf), onesrow.astype(bf))


def make_in_maps(x, T):
    x = np.asarray(x, dtype=np.float32)
    T = np.asarray(T, dtype=np.float32)
    selwb, w8f, onesb, selneg, wpos8, onesrow = _consts()
    f8 = ml_dtypes.float8_e4m3fn
    # [p, kt*n] prearranged layout: row p holds x^T[kt*128 + p, :] for kt=0..7
    xT = np.ascontiguousarray(x.T)                      # [1024, 256]
    xT8 = np.ascontiguousarray(
        xT.reshape(8, P, N).transpose(1, 0, 2).reshape(P, 8 * N)).astype(f8)
    T4 = T.reshape(A_DIM, B, C)
    rpc = N // NCORES
    in_maps = []
    for k in range(NCORES):
        t2g = T4[:, k * BPC:(k + 1) * BPC, :].reshape(A_DIM, BPC * C)
        t2g8 = np.ascontiguousarray(
            t2g.reshape(8, P, BPC * C).transpose(1, 0, 2).reshape(P, -1)).astype(f8)
        in_maps.append({
            "xT8": xT8,
            "t2g8": t2g8,
            "xrows": np.ascontiguousarray(x[k * rpc:(k + 1) * rpc]),
            "selwb": selwb, "w8f": w8f, "onesb": onesb,
            "selneg": selneg, "wpos8": wpos8, "onesrow": onesrow,
        })
    return in_maps


def assemble(results, x):
    full = np.empty((N, A_DIM + B), np.float32)
    rpc = N // NCORES
    for k in range(NCORES):
        full[k * rpc:(k + 1) * rpc, :A_DIM] = results[k]["out_x"]
        acc = np.asarray(results[k]["out_acc"], np.float32)   # [(u b), blk]
        cs = np.asarray(results[k]["out_cs"], np.float32)     # [b, j-16]
        blk = acc.reshape(BLK, BPC, NBLK).transpose(2, 0, 1).reshape(N, BPC) - 1.0
        blk[BLK:, :] += cs.T
        full[:, A_DIM + k * BPC:A_DIM + (k + 1) * BPC] = blk
    return full


def kernel(x, T):
    from concourse.bass_utils import run_bass_kernel_spmd

    if "nc" not in _cache:
        _cache["nc"] = build()
    nc = _cache["nc"]
    in_maps = make_in_maps(x, T)
    prev = os.environ.get("BASS_NEVER_TRACE")
    os.environ["BASS_NEVER_TRACE"] = "1"
    try:
        res = run_bass_kernel_spmd(nc, in_maps, core_ids=list(range(NCORES)))
    finally:
        if prev is None:
            os.environ.pop("BASS_NEVER_TRACE", None)
        else:
            os.environ["BASS_NEVER_TRACE"] = prev
    return assemble(res.results, x)
